# revision 1
# baseline (speedup 1.0000x reference)
"""CrossTransformer (KNN message passing) Trainium2 kernel, v2.

Contract: kernel(**inputs) takes the FULL unsharded inputs (numpy arrays,
keys as in setup_inputs()) and returns the FULL [2, 256, 2048] float32
output.  Internally shards across 8 NeuronCores: core = b*4 + s handles
batch b, key-point shard s (512 points), with the fused KNN database
replicated per core.

Pipeline per core (per 128-point tile):
  1. KNN scores S = 2*k.f - |f|^2 via a 21-row bf16 mantissa-split matmul
     (fp32-grade scores; selected neighbor sets match the fp32 reference)
     copied to SBUF (scalar+pool), exact top-16 via DVE
     max/max_index/match_replace (two top-8 rounds, uint16 indices).
  2. Index tile [128,16] replicated 8x (DVE) and PE-transposed (bf16
     bitcast) into the SWDGE wrapped layout; 4 chunked
     dma_gather(transpose=True) pulls 512 db rows each, laid out
     channel-major j-major: row = [fp8 channel-pairs | feat bf16 | pcd].
     (Requires the gpsimd mlp ucode library; pool must not run
     standard-library tensor ops afterwards.)
  3. pos MLP via PE only: h1p = W1p.pcd_b - W1p.G3 (B-replicator matmul
     for the k-broadcast), pe' = W2p.h1p + feat (featT replicator
     matmul) accumulated in PSUM; scalar copies pe' to fp8 (h1 rhs),
     then +G via identity matmul in the same bank -> V' bf16.
  4. attn MLP in fp8 DoubleRow (K=256 per pass): h1 = W1.pe'8 - W1.G8
     (two streams, weights scaled x64, descaled in the relu), out
     matmul over fp8 h1s, exp on scalar.
  5. Softmax over k (j-major strided fp32 fold adds on DVE), num/den
     divide, out = num/den - feat + b2 (feat was folded into pe').
"""

import copy as _copy

import numpy as np
import ml_dtypes

import concourse.bass as bass
import concourse.mybir as mybir
import concourse.tile as tile
from concourse import bass_utils, library_config
from concourse.masks import make_identity

F32 = mybir.dt.float32
BF16 = mybir.dt.bfloat16
FP8 = mybir.dt.float8e4
U16 = mybir.dt.uint16
I16 = mybir.dt.int16
AF = mybir.ActivationFunctionType
ALU = mybir.AluOpType
DR = mybir.MatmulPerfMode.DoubleRow

B = 2
C = 256
N = 2048
M = 2048
F = N + M            # fused database size
KNN = 16
PH = 64              # pos MLP hidden
AH = 1024            # attn MLP hidden
P = 128
NCORES = 8
SHARD = N * B // NCORES      # 512 key points per core
NT = SHARD // P              # 4 point-tiles per core
ROWU = 512                   # db row in u16 units: 128 fp8-pair + 256 + 3 + pad
CH = 512                     # gather chunk / attn slice (cols)
NCH = P * KNN // CH          # 4 chunks per point-tile
BN_EPS = 1e-5
NEG_BIG = -3.0e38
WS = 64.0                    # fp8 weight scale

# Module-level knobs for test harnesses (not used by the grader).
TRACE = False
LAST_RESULT = None


def _legalize_sync_waits(nc, max_waits=1):
    """walrus here accepts at most one sync wait per instruction; move
    extra waits onto ENGINE_NOP carriers inserted just before the offender
    (same engine: the sequencer accumulates the waits, no pipeline drain)."""
    module = nc.m
    new_module = _copy.replace(module, functions=[])
    for function in module.functions:
        new_function = _copy.replace(function, blocks=[])
        new_function.set_allocations_from_list(function.allocations)
        for block in function.blocks:
            out = []
            for inst in block.instructions:
                si = inst.sync_info
                waits = list(si.on_wait) if si is not None else []
                if len(waits) > max_waits:
                    extra, keep = waits[:-max_waits], waits[-max_waits:]
                    for j in range(0, len(extra), max_waits):
                        out.append(mybir.InstDrain(
                            name=f"I-lgl-{inst.name}-{j}",
                            engine=inst.engine,
                            ins=[], outs=[],
                            sync_info=mybir.SyncInfo(
                                on_wait=extra[j:j + max_waits], on_update=[]),
                        ))
                    inst.sync_info = mybir.SyncInfo(
                        on_wait=keep, on_update=list(si.on_update))
                out.append(inst)
            new_function.blocks.append(_copy.replace(block, instructions=out))
        new_module.functions.append(new_function)
    nc.m = new_module


def _build_bass(legalize=True):
    nc = bass.Bass(num_swdge_queues=4)
    dt = nc.dram_tensor
    keys2t = dt("keys2t", [21, SHARD], BF16, kind="ExternalInput")
    knn_rhs = dt("knn_rhs", [21, F], BF16, kind="ExternalInput")
    db_rows = dt("db_rows", [F, ROWU], BF16, kind="ExternalInput")
    feat_sh = dt("feat_sh", [C, SHARD], F32, kind="ExternalInput")
    featT = dt("featT", [SHARD, C], BF16, kind="ExternalInput")
    pcd_sh = dt("pcd_sh", [4, SHARD], BF16, kind="ExternalInput")
    bmat = dt("bmat", [P, P * KNN], BF16, kind="ExternalInput")
    pos_w1 = dt("pos_w1", [4, PH], BF16, kind="ExternalInput")
    pos_w1n = dt("pos_w1n", [4, PH], BF16, kind="ExternalInput")
    pos_b1 = dt("pos_b1", [PH, 1], F32, kind="ExternalInput")
    pos_w2t = dt("pos_w2t", [PH, C], BF16, kind="ExternalInput")
    pos_b2c = dt("pos_b2c", [P, 2], F32, kind="ExternalInput")
    attn_w18 = dt("attn_w18", [P, 2 * AH], FP8, kind="ExternalInput")
    attn_w1n8 = dt("attn_w1n8", [P, 2 * AH], FP8, kind="ExternalInput")
    attn_b1c = dt("attn_b1c", [P, AH // P], F32, kind="ExternalInput")
    attn_w28 = dt("attn_w28", [P, 2 * AH], FP8, kind="ExternalInput")
    attn_b2c = dt("attn_b2c", [P, 2], F32, kind="ExternalInput")
    out = dt("out", [C, SHARD], F32, kind="ExternalOutput")
    idx_dram = dt("idx_scratch", [NT * P, KNN], U16, kind="Internal")

    with tile.TileContext(nc) as tc:
        with (
            tc.tile_pool(name="const", bufs=1) as cp,
            tc.tile_pool(name="s", bufs=2) as s_pool,
            tc.tile_pool(name="idx", bufs=2) as idx_pool,
            tc.tile_pool(name="g", bufs=3) as g_pool,
            tc.tile_pool(name="pe8", bufs=2) as pe8_pool,
            tc.tile_pool(name="vp", bufs=2) as vp_pool,
            tc.tile_pool(name="h1p", bufs=2) as h1p_pool,
            tc.tile_pool(name="h18", bufs=2) as h18_pool,
            tc.tile_pool(name="ef", bufs=2) as ef_pool,
            tc.tile_pool(name="sm", bufs=2) as sm_pool,
            tc.tile_pool(name="ot", bufs=2) as ot_pool,
            tc.tile_pool(name="ppa", bufs=2, space="PSUM") as pp_aux,
            tc.tile_pool(name="ppe", bufs=2, space="PSUM") as pp_pe,
            tc.tile_pool(name="ppm", bufs=2, space="PSUM") as pp_mlp,
        ):
            # ---- constants / weights ----
            ident = cp.tile([P, P], BF16)
            make_identity(nc, ident[:, :])
            # gpsimd: switch to the mlp ucode library for dma_gather.
            # NOTE: no standard-library pool tensor ops after this point.
            nc.gpsimd.load_library(library_config.mlp)

            keys2t_s = cp.tile([21, SHARD], BF16)
            nc.sync.dma_start(keys2t_s[:, :], keys2t[:, :])
            knn_rhs_s = cp.tile([21, F], BF16)
            nc.sync.dma_start(knn_rhs_s[:, :], knn_rhs[:, :])
            feat_s = cp.tile([P, 2 * SHARD], F32, tag="feat")
            for cc in range(2):
                nc.sync.dma_start(feat_s[:, cc * SHARD:(cc + 1) * SHARD],
                                  feat_sh[cc * P:(cc + 1) * P, :])
            featT_s = []
            for t in range(NT):
                row = []
                for cc in range(2):
                    ftt = cp.tile([P, P], BF16, tag=f"fT{t}_{cc}")
                    nc.sync.dma_start(
                        ftt[:, :],
                        featT[t * P:(t + 1) * P, cc * P:(cc + 1) * P])
                    row.append(ftt)
                featT_s.append(row)
            pcd_s = cp.tile([4, SHARD], BF16)
            nc.sync.dma_start(pcd_s[:, :], pcd_sh[:, :])
            bmat_s = cp.tile([P, P * KNN], BF16)
            nc.sync.dma_start(bmat_s[:, :], bmat[:, :])
            pw1 = cp.tile([4, PH], BF16)
            nc.sync.dma_start(pw1[:, :], pos_w1[:, :])
            pw1n = cp.tile([4, PH], BF16, tag="pw1n")
            nc.sync.dma_start(pw1n[:, :], pos_w1n[:, :])
            pb1 = cp.tile([PH, 1], F32)
            nc.sync.dma_start(pb1[:, :], pos_b1[:, :])
            pw2 = cp.tile([PH, C], BF16)
            nc.sync.dma_start(pw2[:, :], pos_w2t[:, :])
            pb2 = cp.tile([P, 2], F32)
            nc.sync.dma_start(pb2[:, :], pos_b2c[:, :])
            w18 = cp.tile([P, 2 * AH], FP8, tag="w18")
            nc.sync.dma_start(w18[:, :], attn_w18[:, :])
            w1n8 = cp.tile([P, 2 * AH], FP8, tag="w1n8")
            nc.sync.dma_start(w1n8[:, :], attn_w1n8[:, :])
            ab1 = cp.tile([P, AH // P], F32)
            nc.sync.dma_start(ab1[:, :], attn_b1c[:, :])
            w28 = cp.tile([P, 2 * AH], FP8, tag="w28")
            nc.sync.dma_start(w28[:, :], attn_w28[:, :])
            ab2 = cp.tile([P, 2], F32)
            nc.sync.dma_start(ab2[:, :], attn_b2c[:, :])

            g_tiles = [None] * NT
            idx_tiles = [None] * NT
            prep_tiles = [None] * NT

            def phase_knn(t):
                tsl = slice(t * P, (t + 1) * P)
                # KNN scores: S[p, f] = 2*k_p . f - |f|^2 (fp32)
                S = s_pool.tile([P, F], F32, name="S")
                for c in range(F // CH):
                    ps = pp_aux.tile([P, CH], F32, tag="aux", name="ks")
                    nc.tensor.matmul(ps[:, :], lhsT=keys2t_s[:, tsl],
                                     rhs=knn_rhs_s[:, c * CH:(c + 1) * CH],
                                     start=True, stop=True)
                    nc.scalar.activation(S[:, c * CH:(c + 1) * CH],
                                         ps[:, :], AF.Copy)
                # exact top-16: two top-8 rounds (order within 16 is free)
                mx = sm_pool.tile([P, 8], F32, tag="mx", name="mx", bufs=1)
                idx16 = idx_pool.tile([P, KNN], U16, name="idx16", tag="idx16")
                nc.vector.max(out=mx[:, :], in_=S[:, :])
                nc.vector.max_index(idx16[:, 0:8], mx[:, :], S[:, :])
                nc.vector.match_replace(out=S[:, :], in_to_replace=mx[:, :],
                                        in_values=S[:, :], imm_value=NEG_BIG)
                mx2 = sm_pool.tile([P, 8], F32, tag="mx2", name="mx2", bufs=1)
                nc.vector.max(out=mx2[:, :], in_=S[:, :])
                nc.vector.max_index(idx16[:, 8:16], mx2[:, :], S[:, :])
                idx_tiles[t] = idx16

            def phase_gather(t):
                # replicate idx 8x along free (DVE), PE-transpose (bf16 bit
                # move) into the SWDGE wrapped layout [(r 8)(k 16), j 128]
                idx16 = idx_tiles[t]
                idxrep = idx_pool.tile([P, P], U16, name="idxrep",
                                       tag="idxrep")
                nc.gpsimd.tensor_copy(
                    idxrep[:, :].rearrange("p (r k) -> p r k", r=8),
                    idx16[:, :].unsqueeze(1).to_broadcast([P, 8, KNN]))
                idxT_ps = pp_aux.tile([P, P], BF16, tag="aux", name="idxT")
                nc.tensor.transpose(idxT_ps[:, :],
                                    idxrep[:, :].bitcast(BF16), ident[:, :])
                idxs_sb = idx_pool.tile([P, P], U16, name="idxs", tag="idxs")
                nc.vector.tensor_copy(idxs_sb[:, :].bitcast(BF16),
                                      idxT_ps[:, :])
                # G layout: (chunk 4, plane 4, n 512) u16 per partition:
                # plane 0 = fp8 pairs (ch p, ch p+128), planes 1-2 = feat
                # bf16 cc halves, plane 3 = pcd xyz on partitions 0-2.
                G = g_pool.tile([P, NCH * 4 * CH], BF16, name="G")
                for ch in range(NCH):
                    nc.gpsimd.dma_gather(
                        out_ap=G[:, ch * 4 * CH:(ch + 1) * 4 * CH].rearrange(
                            "p (c n) -> p c n", c=4),
                        in_ap=db_rows[:, :],
                        idxs_ap=idxs_sb[:, ch * (CH // KNN):
                                        (ch + 1) * (CH // KNN)].bitcast(I16),
                        num_idxs=CH, num_idxs_reg=CH, elem_size=ROWU,
                        transpose=True, queue_num=ch)
                g_tiles[t] = G

            def phase_prep(t):
                tsl = slice(t * P, (t + 1) * P)
                G = g_tiles[t]

                def gplane(ch, c):
                    return G[:, ch * 4 * CH + c * CH:
                             ch * 4 * CH + (c + 1) * CH]

                # PCT[j, h] = sum_d pcd[d, j] * w1p[h, d]
                pct_ps = pp_aux.tile([P, PH], F32, tag="aux", name="pct")
                nc.tensor.matmul(pct_ps[:, :], lhsT=pcd_s[:, tsl],
                                 rhs=pw1[:, :], start=True, stop=True)
                pct = h1p_pool.tile([P, PH], BF16, tag="pct", name="pct")
                nc.scalar.activation(pct[:, :], pct_ps[:, :], AF.Copy)
                # h1p = relu(PCT.B - W1p.G3 + b1p)   [64, 2048]
                h1p = h1p_pool.tile([PH, P * KNN], BF16, tag="h1p",
                                    name="h1p")
                for cp in range(NCH // 2):
                    hp = pp_mlp.tile([PH, 2 * CH], F32, tag="mm",
                                     name="h1ps")
                    for hh in range(2):
                        ch = cp * 2 + hh
                        hsl = slice(hh * CH, (hh + 1) * CH)
                        nc.tensor.matmul(hp[:, hsl], lhsT=pct[:, :],
                                         rhs=bmat_s[:, ch * CH:(ch + 1) * CH],
                                         start=True, stop=False)
                        nc.tensor.matmul(hp[:, hsl], lhsT=pw1n[:, :],
                                         rhs=gplane(ch, 3)[0:4, :],
                                         start=False, stop=True)
                    nc.scalar.activation(h1p[:, cp * 2 * CH:(cp + 1) * 2 * CH],
                                         hp[:, :], AF.Relu, bias=pb1[:, 0:1])
                # pe' = W2p.h1p + feat (PSUM); -> pe8 fp8; then += G -> V'
                # ch-pairs with both psum tiles alive: one ldweights per
                # lhsT serves two matmuls.
                pe8 = pe8_pool.tile([P, 2 * P * KNN], FP8, name="pe8")
                Vp = vp_pool.tile([P, 2 * P * KNN], BF16, name="vp")
                for cc in range(2):
                    for cp in range(NCH // 2):
                        pps = [pp_pe.tile([P, CH], F32, tag="pe",
                                          name=f"pps{h}") for h in range(2)]
                        for hh in range(2):
                            nc.tensor.matmul(
                                pps[hh][:, :],
                                lhsT=pw2[:, cc * P:(cc + 1) * P],
                                rhs=h1p[:, (cp * 2 + hh) * CH:
                                         (cp * 2 + hh + 1) * CH],
                                start=True, stop=False)
                        for hh in range(2):
                            nc.tensor.matmul(
                                pps[hh][:, :], lhsT=featT_s[t][cc][:, :],
                                rhs=bmat_s[:, (cp * 2 + hh) * CH:
                                           (cp * 2 + hh + 1) * CH],
                                start=False, stop=True)
                        for hh in range(2):
                            sl = slice(cc * P * KNN + (cp * 2 + hh) * CH,
                                       cc * P * KNN + (cp * 2 + hh + 1) * CH)
                            nc.scalar.activation(pe8[:, sl],
                                                 pps[hh][:, :], AF.Copy)
                        for hh in range(2):
                            nc.tensor.matmul(pps[hh][:, :], lhsT=ident[:, :],
                                             rhs=gplane(cp * 2 + hh, 1 + cc),
                                             start=False, stop=True,
                                             skip_group_check=True)
                        for hh in range(2):
                            sl = slice(cc * P * KNN + (cp * 2 + hh) * CH,
                                       cc * P * KNN + (cp * 2 + hh + 1) * CH)
                            if t < 2:
                                nc.scalar.activation(Vp[:, sl], pps[hh][:, :],
                                                     AF.Copy)
                            else:
                                nc.vector.tensor_copy(Vp[:, sl],
                                                      pps[hh][:, :])
                prep_tiles[t] = (pe8, Vp)

            def phase_attn(t):
                tsl = slice(t * P, (t + 1) * P)
                G = g_tiles[t]
                pe8, Vp = prep_tiles[t]
                # DR rhs views: pe8 [128, (s 2: NK), (n)], G8 fp8 pairs
                G8 = G[:, :].bitcast(FP8)
                NK = P * KNN

                def pe8_rhs(ch):
                    return pe8[:, :].rearrange(
                        "p (s n) -> p s n", s=2)[:, :, ch * CH:(ch + 1) * CH]

                def g8_rhs(ch):
                    # chunk ch plane 0 as fp8: [128, (s 2: stride 1),
                    # (n 512: stride 2)] at offset ch*4096
                    return G8[:, ch * 8 * CH:ch * 8 * CH + 2 * CH].rearrange(
                        "p (n s) -> p s n", s=2)

                # h18 layout: (op 4, s 2, n 2048) fp8 per partition
                h18 = h18_pool.tile([P, 4 * 2 * NK], FP8, name="h18", bufs=1)
                for o in range(AH // P):
                    op, s = o // 2, o % 2
                    w18o = w18[:, :].rearrange("p (s h) -> p s h", s=2)[
                        :, :, o * P:(o + 1) * P]
                    w1n8o = w1n8[:, :].rearrange("p (s h) -> p s h", s=2)[
                        :, :, o * P:(o + 1) * P]
                    hps = [pp_mlp.tile([P, 2 * CH], F32, tag="mm",
                                       name=f"hp{cp}")
                           for cp in range(NCH // 2)]
                    for cp in range(NCH // 2):
                        for hh in range(2):
                            nc.tensor.matmul(
                                hps[cp][:, hh * CH:(hh + 1) * CH], lhsT=w18o,
                                rhs=pe8_rhs(cp * 2 + hh),
                                start=True, stop=False, perf_mode=DR)
                    for cp in range(NCH // 2):
                        for hh in range(2):
                            nc.tensor.matmul(
                                hps[cp][:, hh * CH:(hh + 1) * CH], lhsT=w1n8o,
                                rhs=g8_rhs(cp * 2 + hh),
                                start=False, stop=True, perf_mode=DR)
                    for cp in range(NCH // 2):
                        base = op * 2 * NK + s * NK + cp * 2 * CH
                        nc.scalar.activation(
                            h18[:, base:base + 2 * CH],
                            hps[cp][:, :], AF.Relu, scale=1.0 / WS,
                            bias=ab1[:, o:o + 1])
                ef = ef_pool.tile([P, 2 * NK], BF16, name="ef")
                # per-cc: out matmuls + exp, then softmax (overlaps next cc)
                for cc in range(2):
                    lps = [pp_mlp.tile([P, 2 * CH], F32, tag="mm",
                                       name=f"lp{cp}")
                           for cp in range(NCH // 2)]
                    for op in range(4):
                        w28oc = w28[:, :].rearrange(
                            "p (s h) -> p s h", s=2)[
                            :, :, op * C + cc * P:op * C + (cc + 1) * P]
                        h18v = h18[:, op * 2 * NK:(op + 1) * 2 * NK]\
                            .rearrange("p (s n) -> p s n", s=2)
                        for cp in range(NCH // 2):
                            for hh in range(2):
                                ch = cp * 2 + hh
                                nc.tensor.matmul(
                                    lps[cp][:, hh * CH:(hh + 1) * CH],
                                    lhsT=w28oc,
                                    rhs=h18v[:, :, ch * CH:(ch + 1) * CH],
                                    start=(op == 0), stop=(op == 3),
                                    perf_mode=DR)
                    for cp in range(NCH // 2):
                        nc.scalar.activation(
                            ef[:, cc * NK + cp * 2 * CH:
                               cc * NK + (cp + 1) * 2 * CH],
                            lps[cp][:, :], AF.Exp, scale=1.0 / WS,
                            bias=ab2[:, cc:cc + 1])
                    if t == NT - 1:
                        # tail tile: per-cc softmax so cc0's softmax
                        # overlaps cc1's out matmuls
                        efc = ef[:, cc * NK:(cc + 1) * NK]
                        vpc = Vp[:, cc * NK:(cc + 1) * NK]
                        evc = sm_pool.tile([P, NK], BF16, tag="evc",
                                           name="evc", bufs=1)
                        nc.vector.tensor_mul(evc[:, :], efc, vpc)

                        def foldc(srcf, tag):
                            tmp = sm_pool.tile([P, NK // 2], F32, tag="ftc",
                                               name=f"ftc{tag}", bufs=1)
                            t3 = tmp[:, :].rearrange("p (j k) -> p j k", k=8)
                            s3 = srcf.rearrange("p (j k) -> p j k", k=16)
                            nc.vector.tensor_add(t3, s3[:, :, 0:8],
                                                 s3[:, :, 8:16])
                            nc.vector.tensor_add(t3[:, :, 0:4],
                                                 t3[:, :, 0:4],
                                                 t3[:, :, 4:8])
                            nc.vector.tensor_add(t3[:, :, 0:2],
                                                 t3[:, :, 0:2],
                                                 t3[:, :, 2:4])
                            dd = sm_pool.tile([P, P], F32, tag=f"dc{tag}",
                                              name=f"dc{tag}", bufs=1)
                            nc.vector.tensor_add(
                                dd[:, :],
                                t3[:, :, 0:1].rearrange("p j k -> p (j k)"),
                                t3[:, :, 1:2].rearrange("p j k -> p (j k)"))
                            return dd
                        denc = foldc(efc, f"d{cc}")
                        numc = foldc(evc[:, :], f"n{cc}")
                        otc = ot_pool.tile([P, P], F32, tag=f"otc{cc}",
                                           name=f"otc{cc}")
                        rdc = sm_pool.tile([P, P], F32, tag=f"rdc{cc}",
                                           name=f"rdc{cc}", bufs=1)
                        nc.vector.reciprocal(rdc[:, :], denc[:, :])
                        nc.vector.tensor_mul(otc[:, :], numc[:, :],
                                             rdc[:, :])
                        nc.vector.tensor_sub(
                            otc[:, :], otc[:, :],
                            feat_s[:, cc * SHARD + t * P:
                                   cc * SHARD + (t + 1) * P])
                        nc.vector.tensor_scalar(
                            otc[:, :], otc[:, :], pb2[:, cc:cc + 1],
                            None, op0=ALU.add)
                        nc.sync.dma_start(out[cc * P:(cc + 1) * P, tsl],
                                          otc[:, :])
                if t == NT - 1:
                    return
                # fused softmax over k for BOTH cc planes (ef/Vp layout
                # [128, (cc 2, j 128, k 16)]): one fold chain, 256 outputs
                ev = sm_pool.tile([P, 2 * NK], BF16, tag="ev", name="ev",
                                  bufs=1)
                nc.vector.tensor_mul(ev[:, :], ef[:, :], Vp[:, :])

                def foldsum(src3, tag):
                    tmp = sm_pool.tile([P, NK], F32, tag="ftmp",
                                       name=f"ftmp{tag}", bufs=1)
                    t3 = tmp[:, :].rearrange("p (j k) -> p j k", k=8)
                    nc.vector.tensor_add(t3, src3[:, :, 0:8],
                                         src3[:, :, 8:16])
                    nc.vector.tensor_add(t3[:, :, 0:4], t3[:, :, 0:4],
                                         t3[:, :, 4:8])
                    nc.vector.tensor_add(t3[:, :, 0:2], t3[:, :, 0:2],
                                         t3[:, :, 2:4])
                    d = sm_pool.tile([P, 2 * P], F32, tag=f"d{tag}",
                                     name=f"d{tag}", bufs=1)
                    nc.vector.tensor_add(
                        d[:, :],
                        t3[:, :, 0:1].rearrange("p j k -> p (j k)"),
                        t3[:, :, 1:2].rearrange("p j k -> p (j k)"))
                    return d
                den = foldsum(ef[:, :].rearrange("p (j k) -> p j k", k=16),
                              "den")
                num = foldsum(ev[:, :].rearrange("p (j k) -> p j k", k=16),
                              "num")
                out_t = ot_pool.tile([P, 2 * P], F32, tag="ot", name="ot")
                rden = sm_pool.tile([P, 2 * P], F32, tag="rd", name="rd",
                                    bufs=1)
                nc.vector.reciprocal(rden[:, :], den[:, :])
                nc.vector.tensor_mul(out_t[:, :], num[:, :], rden[:, :])
                featv = feat_s[:, :].rearrange(
                    "p (c n) -> p c n", c=2)[:, :, t * P:(t + 1) * P]
                o3 = out_t[:, :].rearrange("p (c j) -> p c j", c=2)
                nc.vector.tensor_sub(o3, o3, featv)
                nc.vector.tensor_add(
                    o3, o3, pb2[:, :].unsqueeze(2).to_broadcast([P, 2, P]))
                for cc in range(2):
                    nc.sync.dma_start(out[cc * P:(cc + 1) * P, tsl],
                                      out_t[:, cc * P:(cc + 1) * P])

            # software pipeline: topks run back-to-back on DVE (softmaxes
            # queue after all topks); PE flows mm -> prep -> attn without
            # idx-transpose stalls (idx goes through a DRAM-DMA roundtrip).
            phase_knn(0)
            phase_knn(1)
            phase_knn(2)
            phase_gather(0)
            phase_prep(0)
            phase_gather(1)
            phase_knn(3)
            phase_prep(1)
            phase_attn(0)
            phase_gather(2)
            phase_prep(2)
            phase_attn(1)
            phase_gather(3)
            phase_prep(3)
            phase_attn(2)
            phase_attn(3)

    mybir.codegen_inst_isa_subclasses(nc)
    if legalize:
        _legalize_sync_waits(nc)
    return nc


_NC = None


def _get_nc():
    global _NC
    if _NC is None:
        _NC = _build_bass()
    return _NC


def _prep_in_maps(pcd, feat, pcd_feadb, feat_feadb,
                  pos_w1, pos_b1, pos_g1, pos_be1, pos_w2, pos_b2,
                  attn_w1, attn_b1, attn_g1, attn_be1, attn_w2, attn_b2):
    f32 = np.float32
    bf16 = ml_dtypes.bfloat16
    fp8 = ml_dtypes.float8_e4m3
    a = {k: np.ascontiguousarray(np.asarray(v), dtype=f32) for k, v in dict(
        pcd=pcd, feat=feat, pcd_feadb=pcd_feadb, feat_feadb=feat_feadb,
        pos_w1=pos_w1, pos_b1=pos_b1, pos_g1=pos_g1, pos_be1=pos_be1,
        pos_w2=pos_w2, pos_b2=pos_b2,
        attn_w1=attn_w1, attn_b1=attn_b1, attn_g1=attn_g1, attn_be1=attn_be1,
        attn_w2=attn_w2, attn_b2=attn_b2).items()}

    fus_pcd = np.concatenate([a['pcd'], a['pcd_feadb']], axis=2)    # [B,3,F]
    fus_feat = np.concatenate([a['feat'], a['feat_feadb']], axis=2)  # [B,C,F]

    # BatchNorm (eval, running stats 0/1) folded into the conv weights.
    sp = (a['pos_g1'].astype(np.float64) / np.sqrt(1.0 + BN_EPS))
    w1p = a['pos_w1'].astype(np.float64) * sp[:, None]
    b1p = a['pos_b1'].astype(np.float64) * sp + a['pos_be1']
    sa = (a['attn_g1'].astype(np.float64) / np.sqrt(1.0 + BN_EPS))
    w1a = a['attn_w1'].astype(np.float64) * sa[:, None]
    # pos_b2 folded into attn bias (pre-relu) and the final output bias.
    b1a = (a['attn_b1'].astype(np.float64) * sa + a['attn_be1']
           + w1a @ a['pos_b2'].astype(np.float64))

    pos_w1_in = np.zeros((4, PH), bf16)
    pos_w1_in[:3] = w1p.T.astype(bf16)
    pos_w1n_in = np.zeros((4, PH), bf16)
    pos_w1n_in[:3] = (-w1p.T).astype(bf16)
    pos_b1v = b1p.astype(f32).reshape(PH, 1)
    pos_w2t = np.ascontiguousarray(a['pos_w2'].T).astype(bf16)
    pos_b2c = np.ascontiguousarray(a['pos_b2'].reshape(2, P).T)

    # fp8 DoubleRow weights, scaled by WS
    w1s = (w1a * WS).astype(f32).astype(fp8)         # [AH, C]
    attn_w18 = np.zeros((P, 2, AH), fp8)
    attn_w1n8 = np.zeros((P, 2, AH), fp8)
    for s in range(2):
        attn_w18[:, s, :] = w1s[:, s * P:(s + 1) * P].T
        attn_w1n8[:, s, :] = (-(w1a * WS)).astype(f32).astype(fp8)[
            :, s * P:(s + 1) * P].T
    attn_b1c = np.ascontiguousarray(b1a.astype(f32).reshape(AH // P, P).T)
    # w2 [C, AH] -> lhsT per (op, s): [p, s*AH + op*C + cc*P + m]
    w2s = (a['attn_w2'].astype(np.float64) * WS).astype(f32).astype(fp8)
    attn_w28 = np.zeros((P, 2, AH), fp8)
    for s in range(2):
        for op in range(4):
            # h-block index h = (op*2+s)*P + p contracted against
            # h18 subtile s of pair op
            attn_w28[:, s, op * C:(op + 1) * C] = (
                w2s[:, (op * 2 + s) * P:(op * 2 + s + 1) * P].T)
    attn_b2c = np.ascontiguousarray(a['attn_b2'].reshape(2, P).T)

    # B replicator: B[j, col] = 1 iff col//16 == j
    bmat = np.zeros((P, P * KNN), bf16)
    cols = np.arange(P * KNN)
    bmat[cols // KNN, cols] = 1.0

    def split3(x):
        h = x.astype(bf16).astype(f32)
        r = x - h
        m = r.astype(bf16).astype(f32)
        return h, m, (r - m).astype(bf16).astype(f32)

    per_batch = []
    for b in range(B):
        # 21-row bf16 split of S[f] = sum_d 2k_d f_d - |f|^2 (see v1)
        fus = fus_pcd[b].astype(f32)
        fh, fm, fl = split3(fus)
        n64 = -np.sum(fus_pcd[b].astype(np.float64) ** 2, axis=0)
        nh, nm, nl = split3(n64.astype(f32) * 0 + n64)
        knn_rhs = np.zeros((21, F), bf16)
        r = 0
        for dd in range(3):
            for row in (fh[dd], fm[dd], fl[dd], fh[dd], fm[dd], fh[dd]):
                knn_rhs[r] = row.astype(bf16)
                r += 1
        knn_rhs[18] = nh.astype(bf16)
        knn_rhs[19] = nm.astype(bf16)
        knn_rhs[20] = nl.astype(bf16)
        # db rows (u16 view): [fp8 pair (ch q | ch q+128) x128 | feat bf16
        # x256 | pcd bf16 x3 | pad]
        f8v = fus_feat[b].astype(fp8)                 # [C, F]
        lo = f8v[:P, :].view(np.uint8).astype(np.uint16)
        hi = f8v[P:, :].view(np.uint8).astype(np.uint16)
        pair_u16 = (lo | (hi << 8))                   # [128, F]
        db = np.zeros((F, ROWU), np.uint16)
        db[:, :P] = pair_u16.T
        db[:, P:P + C] = fus_feat[b].T.astype(bf16).view(np.uint16)
        db[:, P + C:P + C + 3] = fus_pcd[b].T.astype(bf16).view(np.uint16)
        per_batch.append((knn_rhs, np.ascontiguousarray(db).view(bf16)))

    in_maps = []
    for core in range(NCORES):
        b, s = divmod(core, NCORES // B)
        sh = slice(s * SHARD, (s + 1) * SHARD)
        k2 = 2.0 * a['pcd'][b][:, sh].astype(f32)
        kh, km, kl = split3(k2)
        keys2t = np.zeros((21, SHARD), bf16)
        r = 0
        for dd in range(3):
            for krow in (kh[dd], kh[dd], kh[dd], km[dd], km[dd], kl[dd]):
                keys2t[r] = krow.astype(bf16)
                r += 1
        keys2t[18] = 1.0
        keys2t[19] = 1.0
        keys2t[20] = 1.0
        pcd_sh = np.zeros((4, SHARD), bf16)
        pcd_sh[:3] = a['pcd'][b][:, sh].astype(bf16)
        feat_c = np.ascontiguousarray(a['feat'][b][:, sh])
        in_maps.append(dict(
            keys2t=keys2t,
            knn_rhs=per_batch[b][0],
            db_rows=per_batch[b][1],
            feat_sh=feat_c,
            featT=np.ascontiguousarray(feat_c.T).astype(bf16),
            pcd_sh=pcd_sh,
            bmat=bmat,
            pos_w1=pos_w1_in, pos_w1n=pos_w1n_in, pos_b1=pos_b1v,
            pos_w2t=pos_w2t, pos_b2c=pos_b2c,
            attn_w18=attn_w18.reshape(P, 2 * AH),
            attn_w1n8=attn_w1n8.reshape(P, 2 * AH),
            attn_b1c=attn_b1c,
            attn_w28=attn_w28.reshape(P, 2 * AH),
            attn_b2c=attn_b2c,
        ))
    return in_maps


def kernel(**inputs):
    global LAST_RESULT
    nc = _get_nc()
    in_maps = _prep_in_maps(**inputs)
    res = bass_utils.run_bass_kernel_spmd(
        nc, in_maps, core_ids=list(range(NCORES)), trace=TRACE)
    LAST_RESULT = res
    out = np.empty((B, C, N), np.float32)
    for core in range(NCORES):
        b, s = divmod(core, NCORES // B)
        out[b][:, s * SHARD:(s + 1) * SHARD] = res.results[core]["out"]
    return out

